# revision 49
# baseline (speedup 1.0000x reference)
"""Multi-head self-attention (1x1-conv QKV -> softmax attention -> 1x1-conv)
on Trainium2, 8 NeuronCores, data-parallel over (batch, query-half).

Problem (hardcoded): x[4,256,48,48], Wqkv[768,256], bqkv[768], W0[256,256],
b0[256]; heads=8, dim_head=32, n=2304 pixels.

Sharding: core = b*2 + half. Each core computes K/V for its whole image
(2304 keys) and attention + output projection for its 1152 queries.
No cross-core communication.

Per-core dataflow (PE operands bf16, PSUM f32):
  - x_aug [257, 2304] bf16: image (query half permuted first) + ones row.
  - k_all [(m,d)=256, j] bf16 = Wk^T-gathered @ x; bias folded into the
    PSUM->SBUF copy (tensor_scalar add with per-partition bias AP).
  - q_all [(m,d)=256, i] bf16 (Wq, bq pre-scaled by d^-0.5 on host).
  - vT    [j, 8*(32+1)] bf16 = x^T @ Wv_aug: per head 32 v-dims + ones col;
    v bias via the x ones-row (K=1 matmul).
  - scores^T S_T[j, i] = k_m^T q_m per head: K=32 matmuls, head PAIRS packed
    via row tile_position; each matmul output owns a full PSUM bank.
  - P = exp(S_T): split across ScalarE and VectorE so the PE never waits on
    it (HAM clock gate re-throttles the PE to 1.2 GHz if it idles; GpSimd
    has no PSUM port so it can only take SBUF->SBUF epilogue work):
      * ScalarE: table exp (AF.Exp) -> bf16
      * VectorE: Schraudolph bf16 exp — i16 = round(s*128/ln2 +
        (127-0.0436)*128), bitcast int16 -> bf16 gives e^s to ±3 %.
        Sawtooth error is zero-mean-ish and averages out in softmax.
  - out^T+den = [vT | 1]^T @ P: M=33 matmuls col-positioned (0,0)/(0,64)
    into two PSUM banks; accumulation over the 18 key tiles.
    Row 32 (head0) / 96 (head1) = softmax denominators.
  - normalize: den rows copied to partition 0 (GpSimd), reciprocal there
    (custom DVE ops read the input tensor's partition 0), stream_shuffle
    broadcast across the 32-block (DVE), multiply into outc (DVE, bf16).
  - y = W0 @ outc + b0, bias folded into the PSUM->SBUF copy (DVE).
"""

import os as _os

import numpy as np

import concourse.bass as bass
import concourse.mybir as mybir
import concourse.tile as tile
from concourse import bacc
from concourse import bass_utils

F32 = mybir.dt.float32
BF16 = mybir.dt.bfloat16
I16 = mybir.dt.int16
FP8 = mybir.dt.float8e4
AF = mybir.ActivationFunctionType
ALU = mybir.AluOpType

B, C, HH, WW = 4, 256, 48, 48
HEADS, D = 8, 32
N = HH * WW            # 2304 keys per image
NCORES = 8
NQ = N // 2            # 1152 queries per core
JT = N // 128          # 18 key tiles
ICW = 384              # query chunk width (3 chunks per core)
NV = HEADS * (D + 1)   # 264: vT columns (32 v dims + ones col per head)

# Schraudolph bf16 exp constants: i16 = s*A16 + B16, bitcast -> bf16 ~ e^s
A16 = 128.0 / float(np.log(2.0))          # 184.6650
B16 = 128.0 * (127.0 - 0.04365)           # minimax-centered exponent bias

# exp engine split: counts per 18 j-tiles for (ScalarE, VectorE).
# GpSimd has no PSUM port so it cannot help with exp.
_split = _os.environ.get("KSPLIT", "12,6")
NA, ND = (int(t) for t in _split.split(","))
assert NA + ND == JT


def _exp_pattern():
    # Lead with "A" tiles: the DVE queue still holds the previous chunk's
    # epilogue ops at chunk start (strict FIFO), so early DVE exp tiles
    # would stall the PSUM score-buffer release and starve the PE.  The
    # ScalarE queue must stay pure-exp: engine completion semaphores are
    # monotonic counts, so one PV-gated op in the queue transitively gates
    # every later score-buffer release.
    lead = max(0, JT - 2 * ND)
    pat = ["A"] * lead + ["D", "A"] * ND
    return pat[:JT]


EXP_PATTERN = _exp_pattern()


def _chunks(total, step):
    out = []
    o = 0
    while o < total:
        w = min(step, total - o)
        out.append((o, w))
        o += w
    return out


def _body(tc, x_d, wq_d, bq_d, wk_d, bk_d, wv_d, w0_d, w0b_d, y_d):
    from contextlib import ExitStack

    nc = tc.nc
    with ExitStack() as ctx:
        const = ctx.enter_context(tc.tile_pool(name="const", bufs=1))
        data = ctx.enter_context(tc.tile_pool(name="data", bufs=1))

        # ---------------- load inputs ----------------
        # DMA completion semaphores are monotonic counts, so the first
        # projection matmul waits on every DMA issued before its last
        # dependency: issue wk + the first x column chunk first.
        def load2(name, dram, cols, dt=BF16):
            ts_ = [const.tile([128, cols], dt, name=f"{name}{t}", tag=f"{name}{t}") for t in range(2)]
            nc.sync.dma_start(ts_[0][:], dram[0:128, :])
            nc.sync.dma_start(ts_[1][:], dram[128:256, :])
            return ts_

        def load4b(name, dram):
            ts_ = [const.tile([128, 1], F32, name=f"{name}{t}", tag=f"{name}{t}") for t in range(4)]
            for t in range(4):
                nc.sync.dma_start(ts_[t][:], dram[t * 128:(t + 1) * 128, :])
            return ts_

        x_sb = [const.tile([128, N], BF16, name=f"xa{t}", tag=f"xa{t}") for t in range(2)]
        x1_sb = const.tile([1, N], BF16, name="xones", tag="xones")
        wk_sb = load2("wk", wk_d, C)
        bk_sb = load2("bk", bk_d, 1, F32)
        for (o, cw) in _chunks(N, 512):
            nc.sync.dma_start(x_sb[0][:, o:o + cw], x_d[0:128, o:o + cw])
            nc.sync.dma_start(x_sb[1][:, o:o + cw], x_d[128:256, o:o + cw])
        wq_sb = load2("wq", wq_d, C)
        bq_sb = load2("bq", bq_d, 1, F32)
        wv_sb = load2("wv", wv_d, NV)
        wv1_sb = const.tile([1, NV], BF16, name="wvbias", tag="wvbias")
        nc.gpsimd.dma_start(wv1_sb[:], wv_d[256:257, :])
        nc.gpsimd.dma_start(x1_sb[:], x_d[256:257, :])
        w0_sb = load2("w0", w0_d, C)
        w0b_sb = load2("w0b", w0b_d, 1, F32)

        # persistent activations
        k_sb = [data.tile([128, N], BF16, name=f"k{g}", tag=f"k{g}") for g in range(2)]
        q_sb = [data.tile([128, NQ], BF16, name=f"q{g}", tag=f"q{g}") for g in range(2)]
        vt_sb = [data.tile([128, NV], BF16, name=f"vt{j}", tag=f"vt{j}") for j in range(JT)]
        # output tiles in pv layout: tile t = hg*2 + pr holds head 4*hg+2*pr
        # at partitions 0-31 and head 4*hg+2*pr+1 at partitions 64-95
        outc_sb = [data.tile([128, NQ], BF16, name=f"oc{t}", tag=f"oc{t}") for t in range(4)]
        od_sb = [data.tile([128, NQ], BF16, name=f"od{g}", tag=f"od{g}") for g in range(2)]
        y_sb = [data.tile([128, NQ], F32, name=f"y{g}", tag=f"y{g}") for g in range(2)]

        # ---------------- projections ----------------
        # k chunks in x-arrival order first, then q, then v
        with tc.tile_pool(name="prj", bufs=2, space="PSUM") as prj:
            for (o, w) in _chunks(N, 512):
                for hg in range(2):
                    hsl = slice(hg * 128, (hg + 1) * 128)
                    kps = prj.tile([128, 512], F32, name="kps", tag="kps")
                    nc.tensor.matmul(kps[:, :w], wk_sb[0][:, hsl], x_sb[0][:, o:o + w], start=True, stop=False)
                    nc.tensor.matmul(kps[:, :w], wk_sb[1][:, hsl], x_sb[1][:, o:o + w], start=False, stop=True)
                    nc.scalar.activation(k_sb[hg][:, o:o + w], kps[:, :w],
                                         AF.Identity, bias=bk_sb[hg][:, 0:1])
            for (o, w) in _chunks(NQ, 512):
                for hg in range(2):
                    hsl = slice(hg * 128, (hg + 1) * 128)
                    qps = prj.tile([128, 512], F32, name="qps", tag="qps")
                    nc.tensor.matmul(qps[:, :w], wq_sb[0][:, hsl], x_sb[0][:, o:o + w], start=True, stop=False)
                    nc.tensor.matmul(qps[:, :w], wq_sb[1][:, hsl], x_sb[1][:, o:o + w], start=False, stop=True)
                    nc.scalar.activation(q_sb[hg][:, o:o + w], qps[:, :w],
                                         AF.Identity, bias=bq_sb[hg][:, 0:1])
            for j in range(JT):
                jsl = slice(j * 128, (j + 1) * 128)
                vps = prj.tile([128, NV], F32, name="vps", tag="vps")
                nc.tensor.matmul(vps[:], x_sb[0][:, jsl], wv_sb[0][:], start=True, stop=False)
                nc.tensor.matmul(vps[:], x_sb[1][:, jsl], wv_sb[1][:], start=False, stop=False)
                nc.tensor.matmul(vps[:], x1_sb[:, jsl], wv1_sb[:], start=False, stop=True)
                eng = nc.vector if j % 2 == 0 else nc.scalar
                if eng is nc.scalar:
                    nc.scalar.copy(vt_sb[j][:], vps[:])
                else:
                    nc.vector.tensor_copy(vt_sb[j][:], vps[:])

        # ---------------- attention main loop ----------------
        # PSUM budget: st 3 bufs x 2 banks + pv0/pv1 1 buf x 1 bank = 8.
        # 3 score buffers give exp(j) a ~3-tile latency window before the
        # PE's QK(j+3) blocks on the buffer — the in-order PE queue plus the
        # HAM clock gate make every exposed wait a 2x clock penalty.
        with tc.tile_pool(name="stp", bufs=3, space="PSUM") as stp, \
             tc.tile_pool(name="pv0p", bufs=1, space="PSUM") as pv0p, \
             tc.tile_pool(name="pv1p", bufs=1, space="PSUM") as pv1p, \
             tc.tile_pool(name="ptp", bufs=8) as ptp, \
             tc.tile_pool(name="epi", bufs=2) as epi:
            # one-time 1.0 fill: only partition 0 is ever rewritten (recip),
            # partitions 1..31 just satisfy the shuffle's read range
            rt0 = data.tile([32, ICW], F32, name="rt0", tag="rt0")
            rt1 = data.tile([32, ICW], F32, name="rt1", tag="rt1")
            nc.gpsimd.memset(rt0[:], 1.0)
            nc.gpsimd.memset(rt1[:], 1.0)

            # Deferred epilogue: the normalize chain for chunk c is gated on
            # chunk c's last PV matmuls; issuing it inline would park
            # PV-gated ops at the head of the DVE FIFO and stall the next
            # chunk's score-buffer releases.  Instead the PSUM->SBUF copies
            # (which free the pv banks) are issued at j=1 of the NEXT chunk
            # and the normalize chain at j=11, when they are long satisfied.
            def make_epilogue(pv0, pv1, w, ic0, t_idx, last):
                oc = outc_sb[t_idx]
                ocr = epi.tile([128, ICW], F32, name="ocr", tag="ocr")

                def do_ocr():
                    nc.vector.tensor_copy(ocr[0:33, 0:w], pv0[0:33, 0:w])
                    nc.vector.tensor_copy(ocr[64:97, 0:w], pv1[64:97, 0:w])

                def do_rest():
                    # den rows to partition 0 (custom DVE ops read the input
                    # tensor's partition 0 regardless of the AP base)
                    dt0 = epi.tile([1, ICW], F32, name="dt0", tag="dt0")
                    dt1 = epi.tile([1, ICW], F32, name="dt1", tag="dt1")
                    nc.gpsimd.tensor_copy(dt0[0:1, 0:w], ocr[32:33, 0:w])
                    nc.gpsimd.tensor_copy(dt1[0:1, 0:w], ocr[96:97, 0:w])
                    nc.vector.reciprocal_approx_fast(rt0[0:1, 0:w], dt0[0:1, 0:w])
                    nc.vector.reciprocal_approx_fast(rt1[0:1, 0:w], dt1[0:1, 0:w])
                    rr = epi.tile([128, ICW], F32, name="rr", tag="rr")
                    rrb = epi.tile([32, ICW], F32, name="rrb", tag="rrb")
                    nc.vector.stream_shuffle(rr[0:32, 0:w], rt0[0:32, 0:w], [0] * 32)
                    nc.vector.stream_shuffle(rrb[0:32, 0:w], rt1[0:32, 0:w], [0] * 32)
                    nc.gpsimd.tensor_copy(rr[64:96, 0:w], rrb[0:32, 0:w])
                    nc.gpsimd.tensor_mul(oc[0:32, ic0:ic0 + w], ocr[0:32, 0:w], rr[0:32, 0:w])
                    nc.gpsimd.tensor_mul(oc[64:96, ic0:ic0 + w], ocr[64:96, 0:w], rr[64:96, 0:w])
                    if last:
                        # head pair finished: fire its output-remap DMAs
                        g, qr = t_idx // 2, t_idx % 2
                        nc.sync.dma_start(od_sb[g][qr * 64:qr * 64 + 32, :], oc[0:32, :])
                        nc.sync.dma_start(od_sb[g][qr * 64 + 32:qr * 64 + 64, :], oc[64:96, :])

                return do_ocr, do_rest

            pend = None
            for hg in range(2):
                for pr in range(2):
                    rb = pr * 64       # partition base of this head pair
                    t_idx = hg * 2 + pr
                    for (ic0, w) in _chunks(NQ, ICW):
                        pv0 = pv0p.tile([128, ICW], F32, name="pv0", tag="pv0")
                        pv1 = pv1p.tile([128, ICW], F32, name="pv1", tag="pv1")
                        pts = {}

                        def emit_pv(j, w=w, pv0=pv0, pv1=pv1, pts=pts, hg=hg, pr=pr):
                            pt = pts.pop(j)
                            for hl, (pv, base) in enumerate(((pv0, 0), (pv1, 64))):
                                gh = hg * 4 + 2 * pr + hl
                                nc.tensor.matmul(
                                    pv[base:base + 33, 0:w],
                                    vt_sb[j][:, gh * 33:gh * 33 + 33],
                                    pt[:, hl * ICW:hl * ICW + w],
                                    start=(j == 0), stop=(j == JT - 1),
                                    tile_position=(0, base),
                                )

                        for j in range(JT):
                            st = stp.tile([128, 1024], F32, name="st", tag="st")
                            for hl in range(2):
                                nc.tensor.matmul(
                                    st[:, hl * 512:hl * 512 + w],
                                    k_sb[hg][rb + hl * 32:rb + (hl + 1) * 32, j * 128:(j + 1) * 128],
                                    q_sb[hg][rb + hl * 32:rb + (hl + 1) * 32, ic0:ic0 + w],
                                    start=True, stop=True,
                                    tile_position=(rb + hl * 32, 0),
                                )
                            if j == 1 and pend is not None:
                                pend[0]()       # free the previous pv banks
                            pt = ptp.tile([128, 2 * ICW], BF16, name="pt", tag="pt")
                            st_v = st[:].rearrange("p (s q) -> p s q", s=2)[:, :, 0:w]
                            eng = EXP_PATTERN[j]
                            if eng == "A":
                                nc.scalar.activation(
                                    pt[:].rearrange("p (s q) -> p s q", s=2),
                                    st_v, AF.Exp,
                                )
                            else:
                                nc.vector.tensor_scalar(
                                    out=pt[:].bitcast(I16).rearrange("p (s q) -> p s q", s=2),
                                    in0=st_v,
                                    scalar1=A16, scalar2=B16,
                                    op0=ALU.mult, op1=ALU.add,
                                )
                            pts[j] = pt
                            if j == 11 and pend is not None:
                                pend[1]()       # previous chunk's normalize
                                pend = None
                            # 5-tile lookahead: the PV matmul for tile j only
                            # enters the (in-order) PE queue 5 tiles later,
                            # so exp(j) has ~10 matmul slots of latency
                            # budget and the PE never blocks on exp results.
                            if j >= 5:
                                emit_pv(j - 5)
                        for jf in range(JT - 5, JT):
                            emit_pv(jf)
                        pend = make_epilogue(pv0, pv1, w, ic0, t_idx,
                                             last=(ic0 + w == NQ))
            pend[0]()
            pend[1]()

        # ---------------- output projection ----------------
        # outc tiles were already remapped into dense head-major od tiles via
        # SBUF->SBUF DMA as each head pair finished; plain K=128 matmuls here.
        with tc.tile_pool(name="fin", bufs=2, space="PSUM") as fin:
            for mt in range(2):
                msl = slice(mt * 128, (mt + 1) * 128)
                for (o, w) in _chunks(NQ, 512):
                    fps = fin.tile([128, 512], F32, name="fps", tag="fps")
                    nc.tensor.matmul(fps[:, :w], w0_sb[0][:, msl], od_sb[0][:, o:o + w], start=True, stop=False)
                    nc.tensor.matmul(fps[:, :w], w0_sb[1][:, msl], od_sb[1][:, o:o + w], start=False, stop=True)
                    nc.scalar.activation(y_sb[mt][:, o:o + w], fps[:, :w],
                                         AF.Identity, bias=w0b_sb[mt][:, 0:1])
                nc.sync.dma_start(y_d[msl, :], y_sb[mt][:])


def build_program():
    nc = bacc.Bacc(
        "TRN2",
        target_bir_lowering=False,
        debug=False,
        enable_asserts=False,
        num_devices=NCORES,
    )
    x_d = nc.dram_tensor("x", [C + 1, N], BF16, kind="ExternalInput").ap()
    wq_d = nc.dram_tensor("wq", [C, C], BF16, kind="ExternalInput").ap()
    bq_d = nc.dram_tensor("bq", [C, 1], F32, kind="ExternalInput").ap()
    wk_d = nc.dram_tensor("wk", [C, C], BF16, kind="ExternalInput").ap()
    bk_d = nc.dram_tensor("bk", [C, 1], F32, kind="ExternalInput").ap()
    wv_d = nc.dram_tensor("wv", [C + 1, NV], BF16, kind="ExternalInput").ap()
    w0_d = nc.dram_tensor("w0", [C, C], BF16, kind="ExternalInput").ap()
    w0b_d = nc.dram_tensor("w0b", [C, 1], F32, kind="ExternalInput").ap()
    y_d = nc.dram_tensor("y", [C, NQ], F32, kind="ExternalOutput").ap()

    with tile.TileContext(nc) as tc:
        _body(tc, x_d, wq_d, bq_d, wk_d, bk_d, wv_d, w0_d, w0b_d, y_d)
    nc.compile()
    return nc


_CACHE = {}


def _get_program():
    if "nc" not in _CACHE:
        _CACHE["nc"] = build_program()
    return _CACHE["nc"]


def make_in_maps(x, Wqkv, bqkv, W0, b0):
    import ml_dtypes
    f = np.float32
    bf = ml_dtypes.bfloat16
    x = np.asarray(x, f)
    Wqkv = np.asarray(Wqkv, f)
    bqkv = np.asarray(bqkv, f)
    W0 = np.asarray(W0, f)
    b0 = np.asarray(b0, f)

    scale = f(D) ** f(-0.5)
    # channel o = d*24 + k*8 + m ; column layout is head-major (m, d) -> m*32+d
    md = (np.arange(HEADS)[:, None] + 24 * np.arange(D)[None, :]).reshape(-1)
    q_rows, k_rows, v_rows = md + 0, md + 8, md + 16

    wq = np.ascontiguousarray((Wqkv[q_rows, :] * scale).T).astype(bf)
    bq = np.ascontiguousarray((bqkv[q_rows] * scale).reshape(-1, 1), dtype=f)
    wk = np.ascontiguousarray(Wqkv[k_rows, :].T).astype(bf)
    bk = np.ascontiguousarray(bqkv[k_rows].reshape(-1, 1), dtype=f)

    wv = np.zeros((C + 1, NV), f)
    for m in range(HEADS):
        vr = v_rows[m * D:(m + 1) * D]
        wv[0:C, m * 33:m * 33 + 32] = Wqkv[vr, :].T
        wv[C, m * 33:m * 33 + 32] = bqkv[vr]
        wv[C, m * 33 + 32] = 1.0
    wv = wv.astype(bf)

    w0 = np.ascontiguousarray(W0.T).astype(bf)  # [c, o], c rows head-major
    w0b = np.ascontiguousarray(b0.reshape(-1, 1), dtype=f)

    shared = {"wq": wq, "bq": bq, "wk": wk, "bk": bk, "wv": wv, "w0": w0, "w0b": w0b}
    maps = []
    for b in range(B):
        xb = x[b].reshape(C, N)
        for half in range(2):
            if half == 0:
                xp = xb
            else:
                xp = np.concatenate([xb[:, NQ:], xb[:, :NQ]], axis=1)
            x_aug = np.concatenate([xp, np.ones((1, N), f)], axis=0).astype(bf)
            maps.append({"x": np.ascontiguousarray(x_aug), **shared})
    return maps


def assemble_output(ys):
    out = np.empty((B, C, N), np.float32)
    for b in range(B):
        out[b][:, 0:NQ] = ys[2 * b]
        out[b][:, NQ:] = ys[2 * b + 1]
    return out.reshape(B, C, HH, WW)


def run(inputs, trace=False):
    nc = _get_program()
    maps = make_in_maps(**inputs)
    res = bass_utils.run_bass_kernel_spmd(
        nc, maps, core_ids=list(range(NCORES)), trace=trace
    )
    ys = [res.results[c]["y"] for c in range(NCORES)]
    return assemble_output(ys), res.exec_time_ns


def kernel(**inputs):
    out, _ = run(inputs, trace=False)
    return out
